# revision 29
# baseline (speedup 1.0000x reference)
"""DistanceFromAnswerLoss on 8 Trainium2 NeuronCores — fused custom-DVE op, v2.

out = 0.1 * sum_{b,c} mask[b,c] * exp(input[b,c])
  mask[b,c] = |c - t_b| / sqrt(sum_c (c - t_b)^2),  mask = 0 where t_b == 0

Per-core pipeline:
  host    : x cast to bf16; bias_b = ln(0.1) - 0.5*ln(C*(t-mu)^2 + K)
            (-1e4 for t==0); aux also carries t-2048 and t-4096 so every
            fused call indexes the same iota[0:4096] window.
  ScalarE : a dummy [128,1] exp FIRST in program order so the activation
            table loads before the x stream saturates the DMA rings;
            then e' = exp(x + bias_b) -> bf16 per tile.
  VectorE : ONE fused op per tile: out = max(iota - t', t' - iota) * e',
            accum_out = rowsum(out)  — |c-t|*e' and the row reduction in
            a single pass.  Tile 0 is split 2x2048 so the chain starts as
            soon as the gpsimd iota seed lands; the only iota expansion
            ([2048:4096], one 4x tensor_scalar) hides between fused calls.
  TensorE : ones-matmul for the final partition reduce -> [1,1] scalar.
"""

import os
import sys
from contextlib import ExitStack

import numpy as np

sys.path.insert(0, "/opt/trn_rl_repo")

import concourse.bass as bass
import concourse.tile as tile
from concourse import bacc, mybir
from concourse.bass_utils import run_bass_kernel_spmd

B = 4096
C = 8192
N_CORES = 8
ROWS = B // N_CORES
RB = ROWS // 128
W = 4096
NW = C // W
NT = RB * NW                 # 8 tile-equivalents per core
IW = 1024                    # gpsimd iota seed width; DVE-expanded to 4096
NACC = 10                    # one accumulator column per fused call
X_F32 = bool(os.environ.get("X_F32"))
COEFF = 0.1

MU = (C - 1) / 2.0
_S1 = (C - 1) * C // 2
_S2 = (C - 1) * C * (2 * C - 1) // 6
K = float(_S2 - _S1 * _S1 / C)

F32 = mybir.dt.float32
BF16 = mybir.dt.bfloat16
Af = mybir.ActivationFunctionType
Op = mybir.AluOpType
XDT = F32 if X_F32 else BF16

_OP_NAME = "ABSDIFF_MUL_REDUCE_ANT"


def _register_dist_op():
    """out = max(Src0 - s0, s0 - Src0) * Src1 ; accum_out = rowsum(out)."""
    from operator import add

    from concourse import dve_ops
    from concourse.dve_spec import C0, Spec, Src0, Src1, lower, maxx
    from concourse.dve_uop import DveOpSpec

    for op in dve_ops.OPS:
        if op.name == _OP_NAME:
            return op

    def _ref(in0, in1, s0, s1, imm2):
        s = np.asarray(s0, dtype=np.float64)
        d = np.abs(in0.astype(np.float64) - s)
        b = (d * in1.astype(np.float64)).astype(np.float32)
        acc = (
            b.reshape(b.shape[0], -1).astype(np.float64).sum(-1, keepdims=True)
        ).astype(np.float32)
        return b, acc

    spec = Spec(
        body=maxx(Src0 - C0, C0 - Src0) * Src1, accum=add,
        reference=_ref,
    )
    row = dve_ops._CUSTOM_DVE_ROW_BASE + len(dve_ops.OPS)
    shas = {
        ver: DveOpSpec(
            name=_OP_NAME, opcode=row, uops=lower(spec, ver=ver), rd1_en=True
        ).sha(ver)
        for ver in ("v3", "v4")
    }
    op = dve_ops.DveOp(_OP_NAME, spec, subdim=False, uops_sha=shas)
    dve_ops.OPS.append(op)
    dve_ops.CUSTOM_DVE_SPECS[op.name] = spec
    dve_ops._SUB_OPCODE_FOR_NAME[op.name] = row
    return op


# schedule: (rb, c0, width, scalar-group); group g holds t - G_OFF[g] so a
# call of width wd always reads iota[0:wd].  Row-block 0 ramps 1k/1k/2k/4k
# so the fused chain starts as soon as the 1k gpsimd iota seed lands.
G_OFF = (0, 1024, 2048, 4096)


def _schedule():
    s = [(0, 0, 1024, 0), (0, 1024, 1024, 1), (0, 2048, 2048, 2),
         (0, 4096, 4096, 3)]
    for rb in range(1, RB):
        s.append((rb, 0, W, 0))
        s.append((rb, W, W, 3))
    return s


def _build() -> bass.Bass:
    dist_op = _register_dist_op()
    nc = bacc.Bacc("TRN2", target_bir_lowering=False, debug=False)
    x = nc.declare_dram_parameter("x", [RB, 128, C], XDT, isOutput=False)
    # aux columns: [t | t-1024 | t-2048 | t-4096 | bias], 4 rbs each
    aux = nc.declare_dram_parameter("aux", [128, 5 * RB], F32, isOutput=False)
    out = nc.declare_dram_parameter("out", [1, 1], F32, isOutput=True)

    sched = _schedule()

    with tile.TileContext(nc) as tc, ExitStack() as ctx:
        const_pool = ctx.enter_context(tc.tile_pool(name="const", bufs=1))
        xpool = ctx.enter_context(tc.tile_pool(name="x", bufs=5))
        epool = ctx.enter_context(tc.tile_pool(name="e", bufs=4))
        dpool = ctx.enter_context(tc.tile_pool(name="d", bufs=2))
        spool = ctx.enter_context(tc.tile_pool(name="s", bufs=1))
        psum_pool = ctx.enter_context(tc.tile_pool(name="ps", bufs=1, space="PSUM"))

        # --- early triggers on engines whose queues wake ~2us before the
        # Sync preamble finishes: aux from ScalarE, xt0a from GpSimd (both
        # have their own hardware DGE queues, so these transfers also skip
        # the Sync-triggered x packet stream) ------------------------------
        auxt = const_pool.tile([128, 5 * RB], F32)
        nc.scalar.dma_start(out=auxt[:], in_=aux[:, :])

        # --- activation-table preload: dummy exp before the exp stream ---
        dum0 = const_pool.tile([128, 1], F32)
        nc.vector.memset(dum0[:], 0.0)
        dum1 = const_pool.tile([128, 1], BF16)
        nc.scalar.activation(dum1[:], dum0[:], Af.Exp)

        xts = {}
        for si in range(3):
            rb, c0, wd, _ = sched[si]
            xt = xpool.tile([128, wd], XDT, tag=f"x{wd}")
            if si == 0:
                nc.gpsimd.dma_start(out=xt[:], in_=x[rb, :, c0:c0 + wd])
            else:
                nc.sync.dma_start(out=xt[:], in_=x[rb, :, c0:c0 + wd])
            xts[si] = xt

        # ALL per-row scalar tiles are copied on the (otherwise idle)
        # ScalarE queue: the ones the head of the pipeline needs right
        # away here, the rest interleaved between exps in the main loop
        # (each pair costs ~0.6us of Sc slack, never pacing the chain).
        bcols, tg = [], {}
        for rb in range(RB):
            bc = const_pool.tile([128, 1], F32, tag=f"bc{rb}")
            bcols.append(bc)
        used = sorted({(g, rb) for rb, _, _, g in sched})
        for g, rb in used:
            t_ = const_pool.tile([128, 1], F32, tag=f"t{g}_{rb}")
            tg[(g, rb)] = t_

        def sc_copy_bcol(rb):
            nc.scalar.copy(bcols[rb][:], auxt[:, 4 * RB + rb:4 * RB + rb + 1])

        def sc_copy_tg(g, rb):
            nc.scalar.copy(tg[(g, rb)][:], auxt[:, g * RB + rb:g * RB + rb + 1])

        # ScalarE carries only the 4 bias copies (the static scheduler
        # front-loads whatever sits on the Sc queue ahead of exp0, so keep
        # that set minimal); all t copies ride the DVE pre-chain idle window.
        sc_copy_bcol(0)
        deferred = {0: [("b", 1), ("b", 2), ("b", 3)]}
        for g, rb in used:
            nc.vector.tensor_copy(
                tg[(g, rb)][:], auxt[:, g * RB + rb:g * RB + rb + 1]
            )
        ones = const_pool.tile([128, 1], F32)
        nc.vector.memset(ones[:], 1.0)

        acc = spool.tile([128, NACC], F32)

        iota = const_pool.tile([128, W], BF16)
        nc.gpsimd.iota(
            iota[:, 0:IW], pattern=[[1, IW]], base=0, channel_multiplier=0,
            allow_small_or_imprecise_dtypes=True,
        )

        def fused(si, et, rb, width, g):
            dm = dpool.tile([128, width], BF16, tag=f"dm{width}")
            nc.vector._custom_dve(
                dist_op, out=dm[:], in0=iota[:, 0:width], in1=et[:],
                s0=tg[(g, rb)][:], accum_out=acc[:, si:si + 1],
            )

        ets = {}
        for si, (rb, c0, wd, g) in enumerate(sched):
            if si not in xts:
                xt = xpool.tile([128, wd], XDT, tag=f"x{wd}")
                nc.sync.dma_start(out=xt[:], in_=x[rb, :, c0:c0 + wd])
            else:
                xt = xts[si]
            et = epool.tile([128, wd], BF16, tag=f"e{wd}")
            nc.scalar.activation(et[:], xt[:], Af.Exp, bias=bcols[rb][:])
            ets[si] = et
            fused(si, et, rb, wd, g)
            for item in deferred.get(si, ()):
                if item[0] == "b":
                    sc_copy_bcol(item[1])
                else:
                    sc_copy_tg(item[1], item[2])
            # iota doublings hidden between fused calls: [1024:2048] after
            # f1 (first needed by f2), [2048:4096] after f2 (needed by f3)
            if si == 1:
                nc.vector.tensor_scalar(
                    iota[:, IW:2 * IW], iota[:, 0:IW], float(IW), None,
                    op0=Op.add,
                )
            elif si == 2:
                nc.vector.tensor_scalar(
                    iota[:, 2 * IW:4 * IW], iota[:, 0:2 * IW], float(2 * IW),
                    None, op0=Op.add,
                )

        # --- combine: rs = rowsum(acc); tot = ones . rs -> [1,1] ----------
        rs = spool.tile([128, 1], F32)
        nc.vector.tensor_reduce(
            rs[:], acc[:], axis=mybir.AxisListType.X, op=Op.add
        )
        tot_ps = psum_pool.tile([1, 1], F32)
        nc.tensor.matmul(tot_ps[:], ones[:], rs[:], start=True, stop=True)
        tot = spool.tile([1, 1], F32)
        nc.vector.tensor_copy(tot[:], tot_ps[:])
        nc.sync.dma_start(out=out[:, :], in_=tot[:])

    nc.finalize()
    return nc


_NC = None


def _get_nc() -> bass.Bass:
    global _NC
    if _NC is None:
        _NC = _build()
    return _NC


def _to_bf16(a: np.ndarray) -> np.ndarray:
    import ml_dtypes

    return a.astype(ml_dtypes.bfloat16)


def make_in_maps(input: np.ndarray, target: np.ndarray) -> list[dict]:
    x = np.ascontiguousarray(np.asarray(input, dtype=np.float32)).reshape(
        N_CORES, RB, 128, C
    )
    if not X_F32:
        x = _to_bf16(x)
    tf = np.asarray(target).astype(np.float64)
    n2 = C * (tf - MU) ** 2 + K
    bias = np.log(COEFF) - 0.5 * np.log(n2)
    bias = np.where(tf == 0, -1e4, bias).astype(np.float32)
    tv = tf.astype(np.float32).reshape(N_CORES, RB, 128)
    bv = bias.reshape(N_CORES, RB, 128)
    cols = [tv - o for o in G_OFF] + [bv]
    aux = np.concatenate([c.transpose(0, 2, 1) for c in cols], axis=2)
    aux = np.ascontiguousarray(aux, dtype=np.float32)
    return [{"x": x[i], "aux": aux[i]} for i in range(N_CORES)]


def run(input: np.ndarray, target: np.ndarray, trace: bool = False, tmpdir=None):
    nc = _get_nc()
    in_maps = make_in_maps(input, target)
    res = run_bass_kernel_spmd(
        nc, in_maps, list(range(N_CORES)), trace=trace, tmpdir=tmpdir
    )
    total = np.float32(0.0)
    for r in res.results:
        total += np.float32(np.sum(np.asarray(r["out"], dtype=np.float32)))
    return np.asarray(total, dtype=np.float32), res


def kernel(input: np.ndarray, target: np.ndarray) -> np.ndarray:
    out, _ = run(input, target)
    return out
